# revision 37
# baseline (speedup 1.0000x reference)
"""GAT layer (N=8192, D=64) as a Bass/Tile kernel on 8 TRN2 NeuronCores.

Math (reference):
    h  = x @ W.T + b
    s1 = h @ a1 ; s2 = h @ a2                    # [N] each
    score[i,j] = s2[i] + s1[j]
    att = softmax_j(leaky_relu(score))
    out = att @ x

Reformulation used here:
    Fold the linear layer into the attention vectors (host-side constant
    folding of the weights):  v = W.T @ [a1|a2], c12 = b.(a1+a2)
      p1 = x @ v1 ; p2 = x @ v2 ; sh1 = p1 + c12
    Softmax rows are shift invariant, so subtract p2[i] from row i:
      exp(lr(score) - p2[i]) = max( exp(sh1[j]),
                                    exp(0.01*sh1[j]) * exp(-0.99*p2[i]) )
      (lr = leaky-relu; exp is monotone so exp(max(a,b)) = max(exp a, exp b))
    So with per-j-row scalars E1 = exp(sh1), F1 = exp(0.01*sh1) and a
    broadcast tile G2b[j,i] = exp(-0.99*p2[i]), the unnormalized weight
    tile (layout [j partitions, i free]) is ONE tensor_scalar op:
      e[j,i] = max( G2b[j,i] * F1[j],  E1[j] )
    The final matmul (with a ones-column appended to x to get the
    softmax denominator for free) accumulates over j in PSUM:
      outT[0:64, i] += x65[j,:].T @ e[j, i] ; Z[i] = outT[64, i]
    Normalization happens in the [d, i] layout: broadcast Z down the
    partitions with a PE ones-matmul, fast-reciprocal + multiply on the
    broadcast tiles; the host only un-transposes.

Sharding: each core owns N/8 = 1024 query rows i (full x is only a few
MB and is replicated to every core), no collectives. Engine roles:
  DVE   - most of the 64 e-tiles (the pacing stream) + s1 reduces
  ACT   - ~12 e-tiles (two-relu-pass form), exps, epilogue copies
  Pool  - s1 elementwise multiplies, E1 negation
  PE    - 128 accumulating bf16 matmuls, p2 broadcast, Z broadcast

G2b comes from ONE matmul per half: lhsT = v2 replicated to 128 columns
gives out[p, i] = p2[i] on every partition, and the ACT exp reads the
PSUM directly. The two halves use PE row-tiles 0:64 / 64:128 with the
block's x.T stacked [128, 512]. A few dummy matmuls warm the PE out of
its low p-state while the DMAs land.
"""

import sys
import types

import ml_dtypes
import numpy as np

import concourse.bacc as bacc
import concourse.bass as bass
import concourse.mybir as mybir
import concourse.tile as tile
from concourse.bass_utils import run_bass_kernel_spmd


def _install_ntff_hook_shim():
    """The agent image's ``antenv`` lacks ``axon_hooks``; provide it so
    ``run_bass_kernel_spmd(trace=True)`` can capture NTFF profiles."""
    if "antenv.axon_hooks" in sys.modules:
        return
    try:
        from trn_agent_boot.trn_boot import _ntff_profile_via_ctypes

        hook = _ntff_profile_via_ctypes("/opt/axon/libaxon_pjrt.so")
        mod = types.ModuleType("antenv.axon_hooks")
        mod._hook = hook
        mod.get_axon_ntff_profile_hook = lambda: mod._hook
        mod.set_axon_ntff_profile_hook = lambda h: setattr(mod, "_hook", h)
        sys.modules["antenv.axon_hooks"] = mod
    except Exception:
        pass


_install_ntff_hook_shim()

N, D = 8192, 64
NCORES = 8
RB = N // NCORES          # rows (i) per core = 1024
NT = N // 128             # j tiles of 128 = 64
F32 = mybir.dt.float32
F32R = mybir.dt.float32r
F16 = mybir.dt.float16
BF16 = mybir.dt.bfloat16
EXP = mybir.ActivationFunctionType.Exp
RELU = mybir.ActivationFunctionType.Relu
ADD = mybir.AluOpType.add
MUL = mybir.AluOpType.mult
MAX = mybir.AluOpType.max
AX_X = mybir.AxisListType.X
DW = D + 1                # 65: x columns + ones column
PKW = D + 2               # packed width: v1b | c12 | c12s

# j-tiles whose e-tile is computed on the ACT engine (two Relu passes:
# t = relu(G2b*F1 - E1); e = relu(t + E1) == max(G2b*F1, E1) since e > 0).
# Balances the DVE/ACT streams; the Pool engine's Q7 software tensor_scalar
# measured ~15us per tile and is not usable for this.
ACT_TILES = frozenset((3, 5, 11, 19, 21, 27, 35, 37, 43, 45))


def build_bass() -> bass.Bass:
    nc = bacc.Bacc(None)
    # partition-major (p, t, d) layouts, prepared on the host
    xp_d = nc.declare_dram_parameter("xp", [128, NT * D], F32, isOutput=False)
    xbf_d = nc.declare_dram_parameter(
        "xbf", [128, NT * DW], BF16, isOutput=False
    )
    pk_d = nc.declare_dram_parameter("pack", [128, PKW], F32, isOutput=False)
    # block x.T halves stacked [128, 512] | v2 replicated [128, 128], f32r
    xkv_d = nc.declare_dram_parameter("xkv", [128, 640], F32R, isOutput=False)
    out_d = nc.declare_dram_parameter("out", [DW, RB], BF16, isOutput=True)

    with tile.TileContext(nc) as tc:
        with (
            tc.tile_pool(name="persist", bufs=1) as persist,
            tc.tile_pool(name="small", bufs=1) as small,
            tc.tile_pool(name="work", bufs=3) as work,
            tc.tile_pool(name="epool", bufs=20) as epool,
            tc.tile_pool(name="eapool", bufs=4) as eapool,
            tc.tile_pool(name="psumA", bufs=3, space="PSUM") as psumA,
            tc.tile_pool(name="psumW", bufs=1, space="PSUM") as psumW,
            tc.tile_pool(name="psumB", bufs=1, space="PSUM") as psumB,
        ):
            # ---- constants + PE p-state warmup while the DMAs land ----
            ones_bf = small.tile([1, 128], BF16)
            nc.vector.memset(ones_bf, 1.0)
            wscr = small.tile([1, 512], BF16)
            nc.vector.memset(wscr, 0.0)
            w_ps = psumW.tile([128, 512], F32, tag="warm", name="w_ps")
            for _ in range(5):
                nc.tensor.matmul(
                    w_ps, lhsT=ones_bf, rhs=wscr, start=True, stop=True
                )

            # ------- small inputs: ONE packed DMA on the sync queue -------
            pk = small.tile([128, PKW], F32)
            with tc.high_priority():
                nc.sync.dma_start(pk, pk_d[:, :])
            v1b = pk[:, 0:D]             # [128, 64] v1 broadcast down parts
            c12 = pk[:, D : D + 1]       # [128, 1] bias col (c1+c2)
            c12s = pk[:, D + 1 : D + 2]  # [128, 1] 0.01*(c1+c2)

            # ------- x loads: flat contiguous 2D chunks, two queues -------
            xbf_flat = persist.tile([128, NT * DW], BF16)
            x_bf = xbf_flat.rearrange("p (t d) -> p t d", t=NT)
            x_flat = persist.tile([128, NT * D], F32)
            x_sb = x_flat.rearrange("p (t d) -> p t d", t=NT)
            xkv_sb = small.tile([128, 640], F32R)

            def xbf_dma(eng, tw, nw):
                eng.dma_start(
                    xbf_flat[:, tw * DW : (tw + nw) * DW],
                    xbf_d[:, tw * DW : (tw + nw) * DW],
                )

            def xp_dma(eng, tw, nw):
                eng.dma_start(
                    x_flat[:, tw * D : (tw + nw) * D],
                    xp_d[:, tw * D : (tw + nw) * D],
                )

            # earliest-needed tensors on the ACT queue, the rest on SP, in
            # deadline order (descriptor generation is serial per queue)
            with tc.high_priority():
                nc.scalar.dma_start(xkv_sb, xkv_d[:, :])
                xp_dma(nc.sync, 0, 8)
                xbf_dma(nc.sync, 0, 8)
            xp_dma(nc.scalar, 8, 16)
            xbf_dma(nc.sync, 8, 16)
            xp_dma(nc.sync, 24, 20)
            xbf_dma(nc.sync, 24, 20)
            xbf_dma(nc.sync, 44, 20)
            xp_dma(nc.sync, 44, 20)

            # ---------------- p2 for this block -> G2b ----------------
            # One matmul per half: lhsT = v2 replicated 128x makes every out
            # partition the p2 row; ACT exps it straight out of PSUM.
            G2b = persist.tile([128, RB], BF16)
            G2a = persist.tile([128, RB], BF16)
            for h in range(2):
                rows = slice(64 * h, 64 * h + 64)
                pb_ps = psumA.tile([128, 512], F32, tag="ps", name="pb_ps")
                nc.tensor.matmul(
                    pb_ps,
                    lhsT=xkv_sb[rows, 512:640],
                    rhs=xkv_sb[rows, 0:512],
                    start=True,
                    stop=True,
                )
                nc.scalar.activation(
                    out=G2b[:, 512 * h : 512 * (h + 1)],
                    in_=pb_ps,
                    func=EXP,
                    scale=-0.99,
                )
                # second copy so the ACT-relu e-tiles don't contend with the
                # DVE's G2b reads on the same SBUF addresses
                nc.scalar.activation(
                    out=G2a[:, 512 * h : 512 * (h + 1)],
                    in_=pb_ps,
                    func=EXP,
                    scale=-0.99,
                )

            # ---------------- main stream ----------------
            s1c = small.tile([128, NT], F32)
            E1c = small.tile([128, NT], F32)
            F1c = small.tile([128, NT], F32)
            E1n = small.tile([128, NT], F32)
            v1b_b = bass.AP(
                tensor=v1b.tensor,
                offset=v1b.offset,
                ap=[v1b.ap[0], [0, 8], v1b.ap[1]],
            )
            acc0 = psumB.tile([DW, 512], F32, tag="acc0", name="acc0")
            acc1 = psumB.tile([DW, 512], F32, tag="acc1", name="acc1")
            accs = [acc0, acc1]

            tmps = {}

            def s1_mul(c, eng=None):
                # Pool: tmp = x[:, chunk, :] * v1 (broadcast over the 8 tiles)
                # (chunk 0 runs on the then-idle DVE to start the stream early)
                tmps[c] = work.tile([128, 8, D], F32, tag="tmp", name="tmp")
                (eng or nc.gpsimd).tensor_tensor(
                    tmps[c], x_sb[:, 8 * c : 8 * (c + 1), :], v1b_b, op=MUL
                )

            def s1_tail(c):
                # DVE: reduce to s1; ACT: E1 = exp(s1+c12), F1 = exp(.01*(.));
                # Pool: negated E1 for the ACT-relu e-tiles. The wait hint
                # stops the scheduler hoisting later chunks' reduces onto the
                # DVE queue ahead of the first e-tiles.
                sl = slice(8 * c, 8 * (c + 1))
                nc.vector.tensor_reduce(
                    out=s1c[:, sl], in_=tmps[c], axis=AX_X, op=ADD
                )
                nc.scalar.activation(
                    out=E1c[:, sl], in_=s1c[:, sl], func=EXP, bias=c12,
                    scale=1.0,
                )
                nc.scalar.activation(
                    out=F1c[:, sl], in_=s1c[:, sl], func=EXP, bias=c12s,
                    scale=0.01,
                )
                nc.gpsimd.tensor_scalar(
                    out=E1n[:, sl], in0=E1c[:, sl], scalar1=-1.0, scalar2=None,
                    op0=MUL,
                )

            h1_deferred = []

            # software-pipelined: chunk c+1's s1 is produced while chunk c's
            # e-tiles stream, so the DVE never stalls at chunk boundaries
            s1_mul(0, eng=nc.vector)
            s1_mul(1)
            s1_tail(0)
            s1_tail(1)
            for c in range(8):
                if c + 2 < 8:
                    s1_mul(c + 2)
                for jt in range(8 * c, 8 * (c + 1)):
                    if jt in ACT_TILES:
                        # e = relu(relu(G2b*F1 - E1) + E1) on the ACT engine
                        e_t = eapool.tile([128, RB], BF16, tag="ea", name="e_a")
                        t_r = eapool.tile([128, RB], BF16, tag="tr", name="t_r")
                        nc.scalar.activation(
                            out=t_r,
                            in_=G2a,
                            func=RELU,
                            scale=F1c[:, jt : jt + 1],
                            bias=E1n[:, jt : jt + 1],
                        )
                        nc.scalar.activation(
                            out=e_t,
                            in_=t_r,
                            func=RELU,
                            scale=1.0,
                            bias=E1c[:, jt : jt + 1],
                        )
                    else:
                        # e[j,i] = max(G2b[j,i] * F1[j], E1[j]) on DVE
                        e_t = epool.tile([128, RB], BF16, tag="e", name="e_t")
                        nc.vector.tensor_scalar(
                            out=e_t,
                            in0=G2b,
                            scalar1=F1c[:, jt : jt + 1],
                            scalar2=E1c[:, jt : jt + 1],
                            op0=MUL,
                            op1=MAX,
                        )
                    if jt % 8 == 4 and c + 2 < 8:
                        s1_tail(c + 2)
                    if c < 6:
                        for h in range(2):
                            nc.tensor.matmul(
                                accs[h],
                                lhsT=x_bf[:, jt, :],
                                rhs=e_t[:, h * 512 : (h + 1) * 512],
                                start=(jt == 0),
                                stop=False,
                            )
                    else:
                        nc.tensor.matmul(
                            accs[0],
                            lhsT=x_bf[:, jt, :],
                            rhs=e_t[:, 0:512],
                            start=False,
                            stop=(jt == NT - 1),
                        )
                        h1_deferred.append((jt, e_t))

            for jt, e_t in h1_deferred:
                nc.tensor.matmul(
                    accs[1],
                    lhsT=x_bf[:, jt, :],
                    rhs=e_t[:, 512:1024],
                    start=False,
                    stop=(jt == NT - 1),
                )

            # ---------------- epilogue: normalize + store ----------------
            # outT[d, i] / Z[i] without transposes or single-partition DVE
            # ops: broadcast Z down the partitions via a PE ones-matmul, then
            # fast-reciprocal + multiply on the broadcast tiles.
            zrow_bf = small.tile([1, RB], BF16)
            out_sb = small.tile([DW, RB], BF16)
            for h in range(2):
                sl = slice(h * 512, (h + 1) * 512)
                with nc.allow_low_precision(
                    reason="bf16 Z: 0.4% rel err on the softmax denominator"
                ):
                    nc.scalar.copy(
                        out=zrow_bf[:, sl], in_=accs[h][D : D + 1, :]
                    )
                zb_ps = psumA.tile([DW, 512], F32, tag="ps", name="zb_ps")
                nc.tensor.matmul(
                    zb_ps,
                    lhsT=ones_bf[:, 0:DW],
                    rhs=zrow_bf[:, sl],
                    start=True,
                    stop=True,
                )
                rzb = work.tile([DW, 512], F32, tag="rzb", name="rzb")
                nc.vector.reciprocal_approx_fast(rzb, zb_ps)
                nc.vector.tensor_tensor(
                    out=out_sb[:, sl], in0=accs[h][0:DW, :], in1=rzb, op=MUL
                )
                nc.sync.dma_start(out_d[:, sl], out_sb[:, sl])

    nc.finalize()
    return nc


def _execute(inputs: dict, trace: bool = False):
    x = np.ascontiguousarray(np.asarray(inputs["x"], dtype=np.float32))
    W = np.ascontiguousarray(np.asarray(inputs["W"], dtype=np.float32))
    b = np.asarray(inputs["b"], dtype=np.float32).reshape(D)
    a = np.asarray(inputs["a"], dtype=np.float32).reshape(2 * D)
    assert x.shape == (N, D) and W.shape == (D, D)

    # constant-fold the linear layer into the attention vectors
    a1, a2 = a[:D], a[D:]
    v = W.T @ np.stack([a1, a2], axis=1)        # [64, 2]
    c12 = float(b @ a1 + b @ a2)

    # partition-major permutations: (t*128+p, d) -> (p, t*d + e)
    xp = np.ascontiguousarray(
        x.reshape(NT, 128, D).transpose(1, 0, 2).reshape(128, NT * D)
    )
    xe = np.concatenate([x, np.ones((N, 1), np.float32)], axis=1)
    xbf = np.ascontiguousarray(
        xe.reshape(NT, 128, DW)
        .transpose(1, 0, 2)
        .reshape(128, NT * DW)
        .astype(ml_dtypes.bfloat16)
    )
    nc = build_bass()
    pack0 = np.zeros((128, PKW), np.float32)
    pack0[:, 0:D] = v[:, 0][None, :]            # v1 broadcast down partitions
    pack0[:, D] = c12
    pack0[:, D + 1] = 0.01 * c12
    v2rep = np.repeat(v[:, 1][:, None], 128, axis=1)   # [64, 128]
    in_maps = []
    for c in range(NCORES):
        xblkT = x[c * RB : (c + 1) * RB].T      # [64, 1024]
        xkv = np.zeros((128, 640), np.float32)
        xkv[0:64, 0:512] = xblkT[:, 0:512]
        xkv[64:128, 0:512] = xblkT[:, 512:1024]
        xkv[0:64, 512:640] = v2rep
        xkv[64:128, 512:640] = v2rep
        in_maps.append({"xp": xp, "xbf": xbf, "pack": pack0,
                        "xkv": np.ascontiguousarray(xkv)})
    res = run_bass_kernel_spmd(
        nc, in_maps, core_ids=list(range(NCORES)), trace=trace
    )
    # un-transpose each core's output: [65, RB] bf16 -> [RB, 64] f32
    outs = []
    for r in res.results:
        o = np.asarray(r["out"][0:D, :], dtype=np.float32).T
        outs.append(np.ascontiguousarray(o))
    out = np.ascontiguousarray(np.concatenate(outs, axis=0))
    return out, res


def kernel(x, W, b, a):
    out, _ = _execute({"x": x, "W": W, "b": b, "a": a})
    return out


# revision 39
# speedup vs baseline: 1.0896x; 1.0896x over previous
"""GAT layer (N=8192, D=64) as a Bass/Tile kernel on 8 TRN2 NeuronCores.

Math (reference):
    h  = x @ W.T + b
    s1 = h @ a1 ; s2 = h @ a2                    # [N] each
    score[i,j] = s2[i] + s1[j]
    att = softmax_j(leaky_relu(score))
    out = att @ x

Reformulation used here:
    Fold the linear layer into the attention vectors (host-side constant
    folding of the weights):  v = W.T @ [a1|a2], c12 = b.(a1+a2)
      p1 = x @ v1 ; p2 = x @ v2 ; sh1 = p1 + c12
    Softmax rows are shift invariant, so subtract p2[i] from row i:
      exp(lr(score) - p2[i]) = max( exp(sh1[j]),
                                    exp(0.01*sh1[j]) * exp(-0.99*p2[i]) )
      (lr = leaky-relu; exp is monotone so exp(max(a,b)) = max(exp a, exp b))
    So with per-j-row scalars E1 = exp(sh1), F1 = exp(0.01*sh1) and a
    broadcast tile G2b[j,i] = exp(-0.99*p2[i]), the unnormalized weight
    tile (layout [j partitions, i free]) is ONE tensor_scalar op:
      e[j,i] = max( G2b[j,i] * F1[j],  E1[j] )
    The final matmul (with a ones-column appended to x to get the
    softmax denominator for free) accumulates over j in PSUM:
      outT[0:64, i] += x65[j,:].T @ e[j, i] ; Z[i] = outT[64, i]
    Normalization happens in the [d, i] layout: broadcast Z down the
    partitions with a PE ones-matmul, fast-reciprocal + multiply on the
    broadcast tiles; the host only un-transposes.

Sharding: each core owns N/8 = 1024 query rows i (full x is only a few
MB and is replicated to every core), no collectives. Engine roles:
  DVE   - most of the 64 e-tiles (the pacing stream) + s1 reduces
  ACT   - ~12 e-tiles (two-relu-pass form), exps, epilogue copies
  Pool  - s1 elementwise multiplies, E1 negation
  PE    - 128 accumulating bf16 matmuls, p2 broadcast, Z broadcast

G2b comes from ONE matmul per half: lhsT = v2 replicated to 128 columns
gives out[p, i] = p2[i] on every partition, and the ACT exp reads the
PSUM directly. The two halves use PE row-tiles 0:64 / 64:128 with the
block's x.T stacked [128, 512]. A few dummy matmuls warm the PE out of
its low p-state while the DMAs land.
"""

import sys
import types

import ml_dtypes
import numpy as np

import concourse.bacc as bacc
import concourse.bass as bass
import concourse.mybir as mybir
import concourse.tile as tile
from concourse.bass_utils import run_bass_kernel_spmd


def _install_ntff_hook_shim():
    """The agent image's ``antenv`` lacks ``axon_hooks``; provide it so
    ``run_bass_kernel_spmd(trace=True)`` can capture NTFF profiles."""
    if "antenv.axon_hooks" in sys.modules:
        return
    try:
        from trn_agent_boot.trn_boot import _ntff_profile_via_ctypes

        hook = _ntff_profile_via_ctypes("/opt/axon/libaxon_pjrt.so")
        mod = types.ModuleType("antenv.axon_hooks")
        mod._hook = hook
        mod.get_axon_ntff_profile_hook = lambda: mod._hook
        mod.set_axon_ntff_profile_hook = lambda h: setattr(mod, "_hook", h)
        sys.modules["antenv.axon_hooks"] = mod
    except Exception:
        pass


_install_ntff_hook_shim()

N, D = 8192, 64
NCORES = 8
RB = N // NCORES          # rows (i) per core = 1024
NT = N // 128             # j tiles of 128 = 64
F32 = mybir.dt.float32
F32R = mybir.dt.float32r
F16 = mybir.dt.float16
BF16 = mybir.dt.bfloat16
EXP = mybir.ActivationFunctionType.Exp
RELU = mybir.ActivationFunctionType.Relu
ADD = mybir.AluOpType.add
MUL = mybir.AluOpType.mult
MAX = mybir.AluOpType.max
AX_X = mybir.AxisListType.X
DW = D + 1                # 65: x columns + ones column
PKW = D + 2               # packed width: v1b | c12 | c12s

# j-tiles whose e-tile is computed on the ACT engine (two Relu passes:
# t = relu(G2b*F1 - E1); e = relu(t + E1) == max(G2b*F1, E1) since e > 0).
# Balances the DVE/ACT streams; the Pool engine's Q7 software tensor_scalar
# measured ~15us per tile and is not usable for this.
ACT_TILES = frozenset((5, 11, 19, 21, 27, 35, 37, 43, 45, 47))


def build_bass() -> bass.Bass:
    nc = bacc.Bacc(None)
    # partition-major (p, t, d) layouts, prepared on the host
    xp_d = nc.declare_dram_parameter("xp", [128, NT * D], F32, isOutput=False)
    xbf_d = nc.declare_dram_parameter(
        "xbf", [128, NT * DW], BF16, isOutput=False
    )
    pk_d = nc.declare_dram_parameter("pack", [128, PKW], F32, isOutput=False)
    # block x.T halves stacked [128, 512] | v2 replicated [128, 128], f32r
    xkv_d = nc.declare_dram_parameter("xkv", [128, 640], F32R, isOutput=False)
    out_d = nc.declare_dram_parameter("out", [DW, RB], BF16, isOutput=True)

    with tile.TileContext(nc) as tc:
        with (
            tc.tile_pool(name="persist", bufs=1) as persist,
            tc.tile_pool(name="small", bufs=1) as small,
            tc.tile_pool(name="work", bufs=3) as work,
            tc.tile_pool(name="epool", bufs=20) as epool,
            tc.tile_pool(name="eapool", bufs=4) as eapool,
            tc.tile_pool(name="psumA", bufs=3, space="PSUM") as psumA,
            tc.tile_pool(name="psumW", bufs=1, space="PSUM") as psumW,
            tc.tile_pool(name="psumB", bufs=1, space="PSUM") as psumB,
        ):
            # ---- constants + PE p-state warmup while the DMAs land ----
            ones_bf = small.tile([1, 128], BF16)
            nc.vector.memset(ones_bf, 1.0)
            wscr = small.tile([1, 512], BF16)
            nc.vector.memset(wscr, 0.0)
            w_ps = psumW.tile([128, 512], F32, tag="warm", name="w_ps")
            for _ in range(5):
                nc.tensor.matmul(
                    w_ps, lhsT=ones_bf, rhs=wscr, start=True, stop=True
                )

            # ------- small inputs: ONE packed DMA on the sync queue -------
            pk = small.tile([128, PKW], F32)
            with tc.high_priority():
                nc.sync.dma_start(pk, pk_d[:, :])
            v1b = pk[:, 0:D]             # [128, 64] v1 broadcast down parts
            c12 = pk[:, D : D + 1]       # [128, 1] bias col (c1+c2)
            c12s = pk[:, D + 1 : D + 2]  # [128, 1] 0.01*(c1+c2)

            # ------- x loads: flat contiguous 2D chunks, two queues -------
            xbf_flat = persist.tile([128, NT * DW], BF16)
            x_bf = xbf_flat.rearrange("p (t d) -> p t d", t=NT)
            x_flat = persist.tile([128, NT * D], F32)
            x_sb = x_flat.rearrange("p (t d) -> p t d", t=NT)
            xkv_sb = small.tile([128, 640], F32R)

            def xbf_dma(eng, tw, nw):
                eng.dma_start(
                    xbf_flat[:, tw * DW : (tw + nw) * DW],
                    xbf_d[:, tw * DW : (tw + nw) * DW],
                )

            def xp_dma(eng, tw, nw):
                eng.dma_start(
                    x_flat[:, tw * D : (tw + nw) * D],
                    xp_d[:, tw * D : (tw + nw) * D],
                )

            # earliest-needed tensors on the ACT queue, the rest on SP, in
            # deadline order (descriptor generation is serial per queue)
            with tc.high_priority():
                nc.scalar.dma_start(xkv_sb, xkv_d[:, :])
                xp_dma(nc.sync, 0, 8)
                xbf_dma(nc.sync, 0, 8)
            xp_dma(nc.scalar, 8, 16)
            xbf_dma(nc.sync, 8, 16)
            xp_dma(nc.sync, 24, 20)
            xbf_dma(nc.sync, 24, 20)
            xbf_dma(nc.sync, 44, 20)
            xp_dma(nc.sync, 44, 20)

            # ---------------- p2 for this block -> G2b ----------------
            # One matmul per half: lhsT = v2 replicated 128x makes every out
            # partition the p2 row; ACT exps it straight out of PSUM.
            G2b = persist.tile([128, RB], BF16)
            G2a = persist.tile([128, RB], BF16)
            for h in range(2):
                rows = slice(64 * h, 64 * h + 64)
                pb_ps = psumA.tile([128, 512], F32, tag="ps", name="pb_ps")
                nc.tensor.matmul(
                    pb_ps,
                    lhsT=xkv_sb[rows, 512:640],
                    rhs=xkv_sb[rows, 0:512],
                    start=True,
                    stop=True,
                )
                nc.scalar.activation(
                    out=G2b[:, 512 * h : 512 * (h + 1)],
                    in_=pb_ps,
                    func=EXP,
                    scale=-0.99,
                )
                # second copy so the ACT-relu e-tiles don't contend with the
                # DVE's G2b reads on the same SBUF addresses
                nc.scalar.activation(
                    out=G2a[:, 512 * h : 512 * (h + 1)],
                    in_=pb_ps,
                    func=EXP,
                    scale=-0.99,
                )

            # ---------------- main stream ----------------
            s1c = small.tile([128, NT], F32)
            E1c = small.tile([128, NT], F32)
            F1c = small.tile([128, NT], F32)
            E1n = small.tile([128, NT], F32)
            v1b_b = bass.AP(
                tensor=v1b.tensor,
                offset=v1b.offset,
                ap=[v1b.ap[0], [0, 8], v1b.ap[1]],
            )
            acc0 = psumB.tile([DW, 512], F32, tag="acc0", name="acc0")
            acc1 = psumB.tile([DW, 512], F32, tag="acc1", name="acc1")
            accs = [acc0, acc1]

            tmps = {}

            def s1_mul(c, eng=None):
                # Pool: tmp = x[:, chunk, :] * v1 (broadcast over the 8 tiles)
                # (chunk 0 runs on the then-idle DVE to start the stream early)
                tmps[c] = work.tile([128, 8, D], F32, tag="tmp", name="tmp")
                (eng or nc.gpsimd).tensor_tensor(
                    tmps[c], x_sb[:, 8 * c : 8 * (c + 1), :], v1b_b, op=MUL
                )

            def s1_tail(c):
                # DVE: reduce to s1; ACT: E1 = exp(s1+c12), F1 = exp(.01*(.));
                # Pool: negated E1 for the ACT-relu e-tiles. The wait hint
                # stops the scheduler hoisting later chunks' reduces onto the
                # DVE queue ahead of the first e-tiles.
                sl = slice(8 * c, 8 * (c + 1))
                nc.vector.tensor_reduce(
                    out=s1c[:, sl], in_=tmps[c], axis=AX_X, op=ADD
                )
                nc.scalar.activation(
                    out=E1c[:, sl], in_=s1c[:, sl], func=EXP, bias=c12,
                    scale=1.0,
                )
                nc.scalar.activation(
                    out=F1c[:, sl], in_=s1c[:, sl], func=EXP, bias=c12s,
                    scale=0.01,
                )
                nc.gpsimd.tensor_scalar(
                    out=E1n[:, sl], in0=E1c[:, sl], scalar1=-1.0, scalar2=None,
                    op0=MUL,
                )

            h1_deferred = []

            # software-pipelined: chunk c+1's s1 is produced while chunk c's
            # e-tiles stream, so the DVE never stalls at chunk boundaries
            s1_mul(0, eng=nc.vector)
            s1_mul(1)
            s1_tail(0)
            s1_tail(1)
            for c in range(8):
                if c + 2 < 8:
                    s1_mul(c + 2)
                for jt in range(8 * c, 8 * (c + 1)):
                    if jt in ACT_TILES:
                        # e = relu(relu(G2b*F1 - E1) + E1) on the ACT engine
                        e_t = eapool.tile([128, RB], BF16, tag="ea", name="e_a")
                        t_r = eapool.tile([128, RB], BF16, tag="tr", name="t_r")
                        nc.scalar.activation(
                            out=t_r,
                            in_=G2a,
                            func=RELU,
                            scale=F1c[:, jt : jt + 1],
                            bias=E1n[:, jt : jt + 1],
                        )
                        nc.scalar.activation(
                            out=e_t,
                            in_=t_r,
                            func=RELU,
                            scale=1.0,
                            bias=E1c[:, jt : jt + 1],
                        )
                    else:
                        # e[j,i] = max(G2b[j,i] * F1[j], E1[j]) on DVE
                        e_t = epool.tile([128, RB], BF16, tag="e", name="e_t")
                        nc.vector.tensor_scalar(
                            out=e_t,
                            in0=G2b,
                            scalar1=F1c[:, jt : jt + 1],
                            scalar2=E1c[:, jt : jt + 1],
                            op0=MUL,
                            op1=MAX,
                        )
                    if jt % 8 == 4 and c + 2 < 8:
                        s1_tail(c + 2)
                    if c < 7:
                        for h in range(2):
                            nc.tensor.matmul(
                                accs[h],
                                lhsT=x_bf[:, jt, :],
                                rhs=e_t[:, h * 512 : (h + 1) * 512],
                                start=(jt == 0),
                                stop=False,
                            )
                    else:
                        nc.tensor.matmul(
                            accs[0],
                            lhsT=x_bf[:, jt, :],
                            rhs=e_t[:, 0:512],
                            start=False,
                            stop=(jt == NT - 1),
                        )
                        h1_deferred.append((jt, e_t))

            for jt, e_t in h1_deferred:
                nc.tensor.matmul(
                    accs[1],
                    lhsT=x_bf[:, jt, :],
                    rhs=e_t[:, 512:1024],
                    start=False,
                    stop=(jt == NT - 1),
                )

            # ---------------- epilogue: normalize + store ----------------
            # outT[d, i] / Z[i] without transposes or single-partition DVE
            # ops: broadcast Z down the partitions via a PE ones-matmul, then
            # fast-reciprocal + multiply on the broadcast tiles.
            zrow_bf = small.tile([1, RB], BF16)
            out_sb = small.tile([DW, RB], BF16)
            for h in range(2):
                sl = slice(h * 512, (h + 1) * 512)
                with nc.allow_low_precision(
                    reason="bf16 Z: 0.4% rel err on the softmax denominator"
                ):
                    nc.scalar.copy(
                        out=zrow_bf[:, sl], in_=accs[h][D : D + 1, :]
                    )
                zb_ps = psumA.tile([DW, 512], F32, tag="ps", name="zb_ps")
                nc.tensor.matmul(
                    zb_ps,
                    lhsT=ones_bf[:, 0:DW],
                    rhs=zrow_bf[:, sl],
                    start=True,
                    stop=True,
                )
                rzb = work.tile([DW, 512], F32, tag="rzb", name="rzb")
                nc.vector.reciprocal_approx_fast(rzb, zb_ps)
                nc.vector.tensor_tensor(
                    out=out_sb[:, sl], in0=accs[h][0:DW, :], in1=rzb, op=MUL
                )
                nc.sync.dma_start(out_d[:, sl], out_sb[:, sl])

    nc.finalize()
    return nc


def _execute(inputs: dict, trace: bool = False):
    x = np.ascontiguousarray(np.asarray(inputs["x"], dtype=np.float32))
    W = np.ascontiguousarray(np.asarray(inputs["W"], dtype=np.float32))
    b = np.asarray(inputs["b"], dtype=np.float32).reshape(D)
    a = np.asarray(inputs["a"], dtype=np.float32).reshape(2 * D)
    assert x.shape == (N, D) and W.shape == (D, D)

    # constant-fold the linear layer into the attention vectors
    a1, a2 = a[:D], a[D:]
    v = W.T @ np.stack([a1, a2], axis=1)        # [64, 2]
    c12 = float(b @ a1 + b @ a2)

    # partition-major permutations: (t*128+p, d) -> (p, t*d + e)
    xp = np.ascontiguousarray(
        x.reshape(NT, 128, D).transpose(1, 0, 2).reshape(128, NT * D)
    )
    xe = np.concatenate([x, np.ones((N, 1), np.float32)], axis=1)
    xbf = np.ascontiguousarray(
        xe.reshape(NT, 128, DW)
        .transpose(1, 0, 2)
        .reshape(128, NT * DW)
        .astype(ml_dtypes.bfloat16)
    )
    nc = build_bass()
    pack0 = np.zeros((128, PKW), np.float32)
    pack0[:, 0:D] = v[:, 0][None, :]            # v1 broadcast down partitions
    pack0[:, D] = c12
    pack0[:, D + 1] = 0.01 * c12
    v2rep = np.repeat(v[:, 1][:, None], 128, axis=1)   # [64, 128]
    in_maps = []
    for c in range(NCORES):
        xblkT = x[c * RB : (c + 1) * RB].T      # [64, 1024]
        xkv = np.zeros((128, 640), np.float32)
        xkv[0:64, 0:512] = xblkT[:, 0:512]
        xkv[64:128, 0:512] = xblkT[:, 512:1024]
        xkv[0:64, 512:640] = v2rep
        xkv[64:128, 512:640] = v2rep
        in_maps.append({"xp": xp, "xbf": xbf, "pack": pack0,
                        "xkv": np.ascontiguousarray(xkv)})
    res = run_bass_kernel_spmd(
        nc, in_maps, core_ids=list(range(NCORES)), trace=trace
    )
    # un-transpose each core's output: [65, RB] bf16 -> [RB, 64] f32
    outs = []
    for r in res.results:
        o = np.asarray(r["out"][0:D, :], dtype=np.float32).T
        outs.append(np.ascontiguousarray(o))
    out = np.ascontiguousarray(np.concatenate(outs, axis=0))
    return out, res


def kernel(x, W, b, a):
    out, _ = _execute({"x": x, "W": W, "b": b, "a": a})
    return out
